# revision 6
# baseline (speedup 1.0000x reference)
"""Trainium2 Bass kernel for nn_PhaseActionHeads (moe_routing).

Strategy:
  - Pure data parallel over 8 NeuronCores; batch elements (b) are dealt to
    cores balanced per routed head so one shared NEFF fits all cores.
  - Phase routing resolved on the host: each b uses exactly 1 of the 4 heads,
    so only that head's MLP is computed (4x less head compute than dense).
  - Mask compaction: ~50% of (b, a) rows are masked to -10000 in the output;
    only unmasked rows are packed into device columns.
  - The fused[768] input is constant across the 64 actions of a b, so its
    head-layer contribution F[b] = fused[b] @ W1f[head(b)] + b1[head(b)] is
    computed once per b and injected into the per-action pre-activation via a
    one-hot expander matmul (K=32) on the tensor engine.
  - All matmuls in fp16 (1 cycle/row on the PE), fp32 PSUM accumulation.

Device layout is feature-major: [features(partitions), rows(free dim)]; a
"column" is one kept (b, a) pair. Each b owns one "slot"; slots are sorted by
head and grouped into 32-slot windows; per-head slot capacities and per-group
column capacities are shared across cores (runtime-derived, baked per call).
"""

import numpy as np

B, A, FEAT = 2048, 64, 64
EMB, FUS, HID, NH = 192, 768, 384, 4
NCORES = 8
MAXN = 512                   # matmul moving-dim limit
WIN = 32                     # slots per S-window (lhsT partition alignment)

_DT_NP = np.float16
LAST_RESULT = None           # BassKernelResults of the last run (for test.py)


def _plan(head, kept_cnt):
    """Shared-across-cores geometry + per-core slot assignment.

    Returns dict with caps, offsets, group/chunk/piece tables and
    slot_b[core, slot] (-1 = dummy).
    """
    core_bs = [[[] for _ in range(NH)] for _ in range(NCORES)]
    for h in range(NH):
        bs = np.where(head == h)[0]
        bs = bs[np.argsort(-kept_cnt[bs], kind="stable")]
        for i, b in enumerate(bs):
            r, j = divmod(i, NCORES)
            c = j if r % 2 == 0 else NCORES - 1 - j  # snake deal
            core_bs[c][h].append(int(b))
    maxcnt = [max(len(core_bs[c][h]) for c in range(NCORES)) for h in range(NH)]
    caps = [int(-(-m // WIN) * WIN) for m in maxcnt]          # slots per head
    offs = np.concatenate([[0], np.cumsum(caps)]).astype(int)
    S = int(offs[-1])
    G = S // WIN

    slot_b = np.full((NCORES, S), -1, np.int64)
    for c in range(NCORES):
        for h in range(NH):
            o = int(offs[h])
            for i, b in enumerate(core_bs[c][h]):
                slot_b[c, o + i] = b

    # per-group column capacity = max over cores of kept cols in the group
    cap_g = np.zeros(G, np.int64)
    for g in range(G):
        sl = slice(WIN * g, WIN * (g + 1))
        for c in range(NCORES):
            bs = slot_b[c, sl]
            n = int(kept_cnt[bs[bs >= 0]].sum())
            cap_g[g] = max(cap_g[g], n)
    cap_g = -(-cap_g // 8) * 8                                 # pad to 8
    gco = np.concatenate([[0], np.cumsum(cap_g)]).astype(int)  # group col offs
    R = int(gco[-1])

    # head col ranges and balanced <=MAXN chunks (head-aligned)
    head_groups = [range(int(offs[h]) // WIN, int(offs[h + 1]) // WIN)
                   for h in range(NH)]
    chunks = []  # (h, c0, c1, [(p0, p1, g), ...])
    for h in range(NH):
        hg = list(head_groups[h])
        if not hg:
            continue
        hc0, hc1 = int(gco[hg[0]]), int(gco[hg[-1] + 1])
        ncols = hc1 - hc0
        if ncols == 0:
            continue
        nch = -(-ncols // MAXN)
        bounds = [hc0 + (ncols * i) // nch for i in range(nch + 1)]
        for i in range(nch):
            c0, c1 = bounds[i], bounds[i + 1]
            if c0 == c1:
                continue
            pieces = []
            for g in hg:
                p0, p1 = max(c0, int(gco[g])), min(c1, int(gco[g + 1]))
                if p0 < p1:
                    pieces.append((p0, p1, g))
            chunks.append((h, c0, c1, pieces))

    return dict(caps=caps, offs=offs, S=S, G=G, slot_b=slot_b,
                cap_g=cap_g, gco=gco, R=R, chunks=chunks)


def _build_nc(plan, b2_vals):
    import concourse.bacc as bacc
    import concourse.mybir as mybir
    import concourse.tile as tile

    dt = mybir.dt
    DT = dt.float16
    F32 = dt.float32
    Gelu = mybir.ActivationFunctionType.Gelu
    Add = mybir.AluOpType.add

    caps, offs, S, G = plan["caps"], plan["offs"], plan["S"], plan["G"]
    R, chunks = plan["R"], plan["chunks"]

    nc = bacc.Bacc()

    x_af = nc.dram_tensor("af_t", [FEAT, R], DT, kind="ExternalInput")
    x_s = nc.dram_tensor("s_mat", [WIN, R], DT, kind="ExternalInput")
    x_fu = nc.dram_tensor("fused_t", [FUS, S], DT, kind="ExternalInput")
    x_we1 = nc.dram_tensor("we1", [FEAT, EMB], DT, kind="ExternalInput")
    x_we2 = nc.dram_tensor("we2", [EMB, EMB], DT, kind="ExternalInput")
    x_w1e = nc.dram_tensor("w1e", [NH, EMB, HID], DT, kind="ExternalInput")
    x_w1f = nc.dram_tensor("w1f", [NH, FUS, HID], DT, kind="ExternalInput")
    x_w2t = nc.dram_tensor("w2t", [HID, NH], DT, kind="ExternalInput")
    x_b1r = nc.dram_tensor("b1rep", [NH, 128, HID], F32, kind="ExternalInput")
    x_bias = nc.dram_tensor("biaspack", [128, 4], F32, kind="ExternalInput")
    y_out = nc.dram_tensor("out", [1, R], F32, kind="ExternalOutput")

    with tile.TileContext(nc) as tc:
        with (
            tc.tile_pool(name="weights", bufs=1) as wp,
            tc.tile_pool(name="afp", bufs=1) as afp,
            tc.tile_pool(name="work", bufs=3) as work,
            tc.tile_pool(name="hwork", bufs=3) as hwork,
            tc.tile_pool(name="outp", bufs=4) as outp,
            tc.tile_pool(name="ps1", bufs=5, space="PSUM") as pp1,
            tc.tile_pool(name="ps3", bufs=1, space="PSUM") as pp3,
        ):
            # ---- resident loads -------------------------------------------
            af_sb = afp.tile([FEAT, R], DT, name="af")
            nc.sync.dma_start(af_sb[:], x_af[:])
            s_sb = afp.tile([WIN, R], DT, name="smat")
            nc.sync.dma_start(s_sb[:], x_s[:])

            fu_sb = [wp.tile([128, S], DT, name=f"fu{k}") for k in range(6)]
            for k in range(6):
                nc.sync.dma_start(fu_sb[k][:], x_fu[128 * k : 128 * (k + 1), :])

            we1_sb = wp.tile([FEAT, EMB], DT, name="we1")
            nc.sync.dma_start(we1_sb[:], x_we1[:])
            we2_sb = [wp.tile([128, EMB], DT, name="we2k0"),
                      wp.tile([64, EMB], DT, name="we2k1")]
            nc.sync.dma_start(we2_sb[0][:], x_we2[0:128, :])
            nc.sync.dma_start(we2_sb[1][:], x_we2[128:192, :])

            w1e_sb, w1f_sb = {}, {}
            for h in range(NH):
                if caps[h] == 0:
                    continue
                w1e_sb[h] = [wp.tile([128, HID], DT, name=f"w1e{h}k0"),
                             wp.tile([64, HID], DT, name=f"w1e{h}k1")]
                nc.sync.dma_start(w1e_sb[h][0][:], x_w1e[h, 0:128, :])
                nc.sync.dma_start(w1e_sb[h][1][:], x_w1e[h, 128:192, :])
                w1f_sb[h] = [wp.tile([128, HID], DT, name=f"w1f{h}k{k}")
                             for k in range(6)]
                for k in range(6):
                    nc.sync.dma_start(w1f_sb[h][k][:],
                                      x_w1f[h, 128 * k : 128 * (k + 1), :])

            w2t_sb = [wp.tile([128, NH], DT, name=f"w2t{k}") for k in range(3)]
            for k in range(3):
                nc.sync.dma_start(w2t_sb[k][:], x_w2t[128 * k : 128 * (k + 1), :])

            b1r_sb = [wp.tile([128, HID], F32, name=f"b1rep{h}") for h in range(NH)]
            for h in range(NH):
                nc.sync.dma_start(b1r_sb[h][:], x_b1r[h])
            bias_sb = wp.tile([128, 4], F32, name="bias")
            nc.sync.dma_start(bias_sb[:], x_bias[:])

            def bias_ap(col, p):
                return bias_sb[:p, col : col + 1]

            # ---- F stage: FT[slot, :] = fused[slot] @ W1f[h] + b1[h] ------
            ftw = [hwork.tile([WIN, HID], DT, name=f"ftw{g}", bufs=1)
                   for g in range(G)]
            for h in range(NH):
                c = int(caps[h])
                if c == 0:
                    continue
                o = int(offs[h])
                for a0 in range(0, c, 128):
                    m = min(128, c - a0)
                    psf = pp1.tile([m, HID], F32, name="psf", tag="ps1")
                    for k in range(6):
                        nc.tensor.matmul(psf[:], fu_sb[k][:, o + a0 : o + a0 + m],
                                         w1f_sb[h][k][:],
                                         start=(k == 0), stop=(k == 5))
                    # += b1[h] broadcast along slots
                    nc.vector.tensor_tensor(
                        psf[:], psf[:], b1r_sb[h][:m, :], Add)
                    for j in range(m // WIN):
                        g = (o + a0) // WIN + j
                        nc.vector.tensor_copy(ftw[g][:],
                                              psf[WIN * j : WIN * (j + 1), :])

            # ---- main pipeline over column chunks -------------------------
            for h, c0, c1, pieces in chunks:
                N = c1 - c0

                e1_ps = [pp1.tile([128, N], F32, name="e1a", tag="ps1"),
                         pp1.tile([64, N], F32, name="e1b", tag="ps1")]
                nc.tensor.matmul(e1_ps[0][:], we1_sb[:, 0:128],
                                 af_sb[:, c0:c1], start=True, stop=True)
                nc.tensor.matmul(e1_ps[1][:], we1_sb[:, 128:192],
                                 af_sb[:, c0:c1], start=True, stop=True)
                e1 = [work.tile([128, N], DT, name="e1sa", tag="e1sa"),
                      work.tile([64, N], DT, name="e1sb", tag="e1sb")]
                nc.scalar.activation(e1[0][:], e1_ps[0][:], Gelu, bias=bias_ap(0, 128))
                nc.scalar.activation(e1[1][:], e1_ps[1][:], Gelu, bias=bias_ap(1, 64))

                e2_ps = [pp1.tile([128, N], F32, name="e2a", tag="ps1"),
                         pp1.tile([64, N], F32, name="e2b", tag="ps1")]
                for mc, (lo, p) in enumerate([(0, 128), (128, 64)]):
                    nc.tensor.matmul(e2_ps[mc][:], we2_sb[0][:, lo : lo + p],
                                     e1[0][:], start=True, stop=False)
                    nc.tensor.matmul(e2_ps[mc][:], we2_sb[1][:, lo : lo + p],
                                     e1[1][:], start=False, stop=True)
                emb = [work.tile([128, N], DT, name="emba", tag="emba"),
                       work.tile([64, N], DT, name="embb", tag="embb")]
                nc.scalar.activation(emb[0][:], e2_ps[0][:], Gelu, bias=bias_ap(2, 128))
                nc.scalar.activation(emb[1][:], e2_ps[1][:], Gelu, bias=bias_ap(3, 64))

                # head hidden: 3 x [128, N] chunks, one bank each inside a
                # 3-bank psum tile (a matmul output cannot cross banks)
                hps = pp3.tile([128, 3, MAXN], F32, name="hps", tag="ps3")
                for m in range(3):
                    sl = hps[:, m, :N]
                    nc.tensor.matmul(sl, w1e_sb[h][0][:, 128 * m : 128 * (m + 1)],
                                     emb[0][:], start=True, stop=False)
                    nc.tensor.matmul(sl, w1e_sb[h][1][:, 128 * m : 128 * (m + 1)],
                                     emb[1][:], start=False, stop=False)
                    for i, (p0, p1, g) in enumerate(pieces):
                        nc.tensor.matmul(
                            hps[:, m, p0 - c0 : p1 - c0],
                            ftw[g][:, 128 * m : 128 * (m + 1)],
                            s_sb[:, p0:p1],
                            start=False, stop=(i == len(pieces) - 1))
                h_sb = hwork.tile([128, 3, MAXN], DT, name="hsb", tag="hsb")
                nc.scalar.activation(h_sb[:], hps[:], Gelu)

                w2_ps = pp1.tile([1, N], F32, name="w2ps", tag="ps1")
                for k in range(3):
                    nc.tensor.matmul(w2_ps[:], w2t_sb[k][:, h : h + 1],
                                     h_sb[:, k, :N],
                                     start=(k == 0), stop=(k == 2))
                o_sb = outp.tile([1, N], F32, name="osb", tag="osb")
                nc.vector.tensor_scalar_add(o_sb[:], w2_ps[:], float(b2_vals[h]))
                nc.sync.dma_start(y_out[:, c0:c1], o_sb[:])

    nc.finalize()
    return nc


def kernel(action_feats, action_mask, fused, phase_oh,
           We1, be1, We2, be2, W1, b1, W2, b2):
    global LAST_RESULT
    from concourse.bass_utils import run_bass_kernel_spmd

    action_feats = np.asarray(action_feats, dtype=np.float32)
    action_mask = np.asarray(action_mask).astype(bool)
    fused = np.asarray(fused, dtype=np.float32)
    phase_oh = np.asarray(phase_oh, dtype=np.float32)
    We1 = np.asarray(We1, dtype=np.float32)
    be1 = np.asarray(be1, dtype=np.float32)
    We2 = np.asarray(We2, dtype=np.float32)
    be2 = np.asarray(be2, dtype=np.float32)
    W1 = np.asarray(W1, dtype=np.float32)
    b1 = np.asarray(b1, dtype=np.float32)
    W2 = np.asarray(W2, dtype=np.float32)
    b2 = np.asarray(b2, dtype=np.float32)

    head = np.minimum(np.argmax(phase_oh, axis=-1), NH - 1)     # [B]
    kept = ~action_mask                                         # [B, A]
    kept_cnt = kept.sum(axis=1).astype(np.int64)
    kept_a = [np.flatnonzero(kept[b]) for b in range(B)]

    plan = _plan(head, kept_cnt)
    S, G, R = plan["S"], plan["G"], plan["R"]
    slot_b, gco = plan["slot_b"], plan["gco"]

    bias_pack = np.zeros((128, 4), np.float32)
    bias_pack[:, 0] = be1[0:128]
    bias_pack[0:64, 1] = be1[128:192]
    bias_pack[:, 2] = be2[0:128]
    bias_pack[0:64, 3] = be2[128:192]

    shared = {
        "we1": We1.astype(_DT_NP),
        "we2": We2.astype(_DT_NP),
        "w1e": np.ascontiguousarray(W1[:, :EMB, :]).astype(_DT_NP),
        "w1f": np.ascontiguousarray(W1[:, EMB:, :]).astype(_DT_NP),
        "w2t": np.ascontiguousarray(W2.T).astype(_DT_NP),
        "b1rep": np.ascontiguousarray(np.broadcast_to(b1[:, None, :], (NH, 128, HID))),
        "biaspack": bias_pack,
    }

    in_maps = []
    col_maps = np.full((NCORES, R), -1, np.int64)
    for c in range(NCORES):
        af_t = np.zeros((FEAT, R), _DT_NP)
        s_mat = np.zeros((WIN, R), _DT_NP)
        fu_t = np.zeros((FUS, S), _DT_NP)
        for g in range(G):
            cur = int(gco[g])
            for s in range(WIN * g, WIN * (g + 1)):
                b = slot_b[c, s]
                if b < 0:
                    continue
                fu_t[:, s] = fused[b]
                aa = kept_a[b]
                n = len(aa)
                if n:
                    af_t[:, cur : cur + n] = action_feats[b, aa, :].T
                    s_mat[s - WIN * g, cur : cur + n] = 1.0
                    col_maps[c, cur : cur + n] = b * A + aa
                    cur += n
        in_maps.append({"af_t": af_t, "s_mat": s_mat, "fused_t": fu_t, **shared})

    nc = _build_nc(plan, b2)
    res = run_bass_kernel_spmd(nc, in_maps, core_ids=list(range(NCORES)))
    LAST_RESULT = res

    logits = np.where(action_mask, np.float32(-10000.0),
                      np.float32(0.0)).reshape(-1)
    for c in range(NCORES):
        out = res.results[c]["out"][0]          # [R]
        valid = col_maps[c] >= 0
        logits[col_maps[c][valid]] = out[valid]
    return logits.reshape(B, A)


# revision 9
# speedup vs baseline: 1.7810x; 1.7810x over previous
"""Trainium2 Bass kernel for nn_PhaseActionHeads (moe_routing).

Strategy:
  - Pure data parallel over 8 NeuronCores; batch elements (b) are dealt to
    cores balanced per routed head so one shared NEFF fits all cores.
  - Phase routing resolved on the host: each b uses exactly 1 of the 4 heads,
    so only that head's MLP is computed (4x less head compute than dense).
  - Mask compaction: ~50% of (b, a) rows are masked to -10000 in the output;
    only unmasked rows are packed into device columns.
  - The fused[768] input is constant across the 64 actions of a b, so its
    head-layer contribution F[b] = fused[b] @ W1f[head(b)] + b1[head(b)] is
    computed once per b and injected into the per-action pre-activation via a
    one-hot expander matmul (K=32) on the tensor engine.
  - All matmuls in fp16 (1 cycle/row on the PE), fp32 PSUM accumulation.

Device layout is feature-major: [features(partitions), rows(free dim)]; a
"column" is one kept (b, a) pair. Each b owns one "slot"; slots are sorted by
head and grouped into 32-slot windows; per-head slot capacities and per-group
column capacities are shared across cores (runtime-derived, baked per call).
"""

import numpy as np

B, A, FEAT = 2048, 64, 64
EMB, FUS, HID, NH = 192, 768, 384, 4
NCORES = 8
MAXN = 512                   # matmul moving-dim limit
WIN = 32                     # slots per S-window (lhsT partition alignment)

_DT_NP = np.float16
LAST_RESULT = None           # BassKernelResults of the last run (for test.py)


def _plan(head, kept_cnt):
    """Shared-across-cores geometry + per-core slot assignment.

    Returns dict with caps, offsets, group/chunk/piece tables and
    slot_b[core, slot] (-1 = dummy).
    """
    core_bs = [[[] for _ in range(NH)] for _ in range(NCORES)]
    for h in range(NH):
        bs = np.where(head == h)[0]
        bs = bs[np.argsort(-kept_cnt[bs], kind="stable")]
        for i, b in enumerate(bs):
            r, j = divmod(i, NCORES)
            c = j if r % 2 == 0 else NCORES - 1 - j  # snake deal
            core_bs[c][h].append(int(b))
    maxcnt = [max(len(core_bs[c][h]) for c in range(NCORES)) for h in range(NH)]
    caps = [int(-(-m // WIN) * WIN) for m in maxcnt]          # slots per head
    offs = np.concatenate([[0], np.cumsum(caps)]).astype(int)
    S = int(offs[-1])
    G = S // WIN

    slot_b = np.full((NCORES, S), -1, np.int64)
    for c in range(NCORES):
        for h in range(NH):
            o = int(offs[h])
            for i, b in enumerate(core_bs[c][h]):
                slot_b[c, o + i] = b

    # per-group column capacity = max over cores of kept cols in the group
    cap_g = np.zeros(G, np.int64)
    for g in range(G):
        sl = slice(WIN * g, WIN * (g + 1))
        for c in range(NCORES):
            bs = slot_b[c, sl]
            n = int(kept_cnt[bs[bs >= 0]].sum())
            cap_g[g] = max(cap_g[g], n)
    cap_g = -(-cap_g // 8) * 8                                 # pad to 8
    gco = np.concatenate([[0], np.cumsum(cap_g)]).astype(int)  # group col offs
    R = int(gco[-1])

    # head col ranges and balanced <=MAXN chunks (head-aligned)
    head_groups = [range(int(offs[h]) // WIN, int(offs[h + 1]) // WIN)
                   for h in range(NH)]
    chunks = []  # (h, c0, c1, [(p0, p1, g), ...])
    for h in range(NH):
        hg = list(head_groups[h])
        if not hg:
            continue
        hc0, hc1 = int(gco[hg[0]]), int(gco[hg[-1] + 1])
        ncols = hc1 - hc0
        if ncols == 0:
            continue
        nch = -(-ncols // MAXN)
        bounds = [hc0 + (ncols * i) // nch for i in range(nch + 1)]
        for i in range(nch):
            c0, c1 = bounds[i], bounds[i + 1]
            if c0 == c1:
                continue
            pieces = []
            for g in hg:
                p0, p1 = max(c0, int(gco[g])), min(c1, int(gco[g + 1]))
                if p0 < p1:
                    pieces.append((p0, p1, g))
            chunks.append((h, c0, c1, pieces))

    return dict(caps=caps, offs=offs, S=S, G=G, slot_b=slot_b,
                cap_g=cap_g, gco=gco, R=R, chunks=chunks)


def _build_nc(plan, b2_vals):
    import concourse.bacc as bacc
    import concourse.mybir as mybir
    import concourse.tile as tile

    dt = mybir.dt
    DT = dt.float16
    F32 = dt.float32
    Gelu = mybir.ActivationFunctionType.Gelu
    Add = mybir.AluOpType.add

    caps, offs, S, G = plan["caps"], plan["offs"], plan["S"], plan["G"]
    R, chunks = plan["R"], plan["chunks"]

    nc = bacc.Bacc()

    x_af = nc.dram_tensor("af_t", [FEAT, R], DT, kind="ExternalInput")
    x_s = nc.dram_tensor("s_mat", [WIN, R], DT, kind="ExternalInput")
    x_fu = nc.dram_tensor("fused_t", [FUS, S], DT, kind="ExternalInput")
    x_we1 = nc.dram_tensor("we1", [FEAT, EMB], DT, kind="ExternalInput")
    x_we2 = nc.dram_tensor("we2", [EMB, EMB], DT, kind="ExternalInput")
    x_w1e = nc.dram_tensor("w1e", [NH, EMB, HID], DT, kind="ExternalInput")
    x_w1f = nc.dram_tensor("w1f", [NH, FUS, HID], DT, kind="ExternalInput")
    x_w2t = nc.dram_tensor("w2t", [HID, NH], DT, kind="ExternalInput")
    x_bias = nc.dram_tensor("biaspack", [128, 16], F32, kind="ExternalInput")
    y_out = nc.dram_tensor("out", [1, R], F32, kind="ExternalOutput")

    with tile.TileContext(nc) as tc:
        with (
            tc.tile_pool(name="weights", bufs=1) as wp,
            tc.tile_pool(name="afp", bufs=1) as afp,
            tc.tile_pool(name="work", bufs=8) as work,
            tc.tile_pool(name="hwork", bufs=8) as hwork,
            tc.tile_pool(name="outp", bufs=4) as outp,
            tc.tile_pool(name="ps1", bufs=8, space="PSUM") as pp1,
        ):
            # ---- resident loads -------------------------------------------
            af_sb = afp.tile([FEAT, R], DT, name="af")
            nc.sync.dma_start(af_sb[:], x_af[:])
            s_sb = afp.tile([WIN, R], DT, name="smat")
            nc.sync.dma_start(s_sb[:], x_s[:])

            fu_sb = [wp.tile([128, S], DT, name=f"fu{k}") for k in range(6)]
            for k in range(6):
                nc.sync.dma_start(fu_sb[k][:], x_fu[128 * k : 128 * (k + 1), :])

            we1_sb = wp.tile([FEAT, EMB], DT, name="we1")
            nc.sync.dma_start(we1_sb[:], x_we1[:])
            we2_sb = [wp.tile([128, EMB], DT, name="we2k0"),
                      wp.tile([64, EMB], DT, name="we2k1")]
            nc.sync.dma_start(we2_sb[0][:], x_we2[0:128, :])
            nc.sync.dma_start(we2_sb[1][:], x_we2[128:192, :])

            w1e_sb, w1f_sb = {}, {}
            for h in range(NH):
                if caps[h] == 0:
                    continue
                w1e_sb[h] = [wp.tile([128, HID], DT, name=f"w1e{h}k0"),
                             wp.tile([64, HID], DT, name=f"w1e{h}k1")]
                nc.sync.dma_start(w1e_sb[h][0][:], x_w1e[h, 0:128, :])
                nc.sync.dma_start(w1e_sb[h][1][:], x_w1e[h, 128:192, :])
                w1f_sb[h] = [wp.tile([128, HID], DT, name=f"w1f{h}k{k}")
                             for k in range(6)]
                for k in range(6):
                    nc.sync.dma_start(w1f_sb[h][k][:],
                                      x_w1f[h, 128 * k : 128 * (k + 1), :])

            w2t_sb = [wp.tile([128, NH], DT, name=f"w2t{k}") for k in range(3)]
            for k in range(3):
                nc.sync.dma_start(w2t_sb[k][:], x_w2t[128 * k : 128 * (k + 1), :])

            bias_sb = wp.tile([128, 16], F32, name="bias")
            nc.sync.dma_start(bias_sb[:], x_bias[:])

            def bias_ap(col, p):
                return bias_sb[:p, col : col + 1]

            # ---- F stage: FT[slot, :] = fused[slot] @ W1f[h] + b1[h] ------
            ftw = [hwork.tile([WIN, HID], DT, name=f"ftw{g}", bufs=1)
                   for g in range(G)]
            for h in range(NH):
                c = int(caps[h])
                if c == 0:
                    continue
                o = int(offs[h])
                for a0 in range(0, c, 128):
                    m = min(128, c - a0)
                    psf = pp1.tile([m, HID], F32, name="psf", tag="ps1")
                    for k in range(6):
                        nc.tensor.matmul(psf[:], fu_sb[k][:, o + a0 : o + a0 + m],
                                         w1f_sb[h][k][:],
                                         start=(k == 0), stop=(k == 5))
                    for j in range(m // WIN):
                        g = (o + a0) // WIN + j
                        nc.vector.tensor_copy(ftw[g][:],
                                              psf[WIN * j : WIN * (j + 1), :])

            # ---- main pipeline: stage-major per head ----------------------
            # Consecutive matmuls share the stationary operand (one LDWEIGHTS
            # per weight, FWL-friendly) and the PE stays dense/warm.
            by_head = {}
            for h, c0, c1, pieces in chunks:
                by_head.setdefault(h, []).append((c0, c1, pieces))

            batches = []
            for h in sorted(by_head):
                hc = by_head[h]
                for i in range(0, len(hc), 4):
                    batches.append((h, hc[i : i + 4]))
            for h, hchunks in batches:
                e1_ps, e1, e2_ps, emb, h_ps, h_sb = {}, {}, {}, {}, {}, {}

                # E1: 2 weight sets, sweep chunks inside
                for mc, (lo, p) in enumerate([(0, 128), (128, 64)]):
                    for t, (c0, c1, _) in enumerate(hchunks):
                        ps = pp1.tile([p, c1 - c0], F32, name=f"e1p{mc}", tag="ps1")
                        nc.tensor.matmul(ps[:], we1_sb[:, lo : lo + p],
                                         af_sb[:, c0:c1], start=True, stop=True)
                        e1_ps[mc, t] = ps
                for t, (c0, c1, _) in enumerate(hchunks):
                    for mc, p in enumerate((128, 64)):
                        s = work.tile([p, c1 - c0], DT, name=f"e1s{mc}", tag=f"e1s{mc}")
                        nc.scalar.activation(s[:], e1_ps[mc, t][:], Gelu,
                                             bias=bias_ap(mc, p))
                        e1[mc, t] = s

                # E2: 4 weight sets
                for mc, (lo, p) in enumerate([(0, 128), (128, 64)]):
                    for kt in range(2):
                        for t, (c0, c1, _) in enumerate(hchunks):
                            if kt == 0:
                                ps = pp1.tile([p, c1 - c0], F32, name=f"e2p{mc}",
                                              tag="ps1")
                                e2_ps[mc, t] = ps
                            nc.tensor.matmul(e2_ps[mc, t][:],
                                             we2_sb[kt][:, lo : lo + p],
                                             e1[kt, t][:],
                                             start=(kt == 0), stop=(kt == 1))
                for t, (c0, c1, _) in enumerate(hchunks):
                    for mc, p in enumerate((128, 64)):
                        s = work.tile([p, c1 - c0], DT, name=f"es{mc}", tag=f"es{mc}")
                        nc.scalar.activation(s[:], e2_ps[mc, t][:], Gelu,
                                             bias=bias_ap(2 + mc, p))
                        emb[mc, t] = s

                # H: per hid-chunk, 2 emb k-tiles + S pieces; 3 separate psums
                for m in range(3):
                    for kt in range(2):
                        for t, (c0, c1, _) in enumerate(hchunks):
                            if kt == 0:
                                ps = pp1.tile([128, c1 - c0], F32, name=f"hp{m}",
                                              tag="ps1")
                                h_ps[m, t] = ps
                            nc.tensor.matmul(h_ps[m, t][:],
                                             w1e_sb[h][kt][:, 128 * m : 128 * (m + 1)],
                                             emb[kt, t][:],
                                             start=(kt == 0), stop=False)
                    for t, (c0, c1, pieces) in enumerate(hchunks):
                        for i, (p0, p1, g) in enumerate(pieces):
                            nc.tensor.matmul(
                                h_ps[m, t][:, p0 - c0 : p1 - c0],
                                ftw[g][:, 128 * m : 128 * (m + 1)],
                                s_sb[:, p0:p1],
                                start=False, stop=(i == len(pieces) - 1))
                    for t, (c0, c1, _) in enumerate(hchunks):
                        s = hwork.tile([128, c1 - c0], DT, name=f"hs{m}", tag=f"hs{m}")
                        nc.scalar.activation(s[:], h_ps[m, t][:], Gelu,
                                             bias=bias_ap(4 + h * 3 + m, 128))
                        h_sb[m, t] = s

                # W2: 3 weight sets
                w2_ps = {}
                for k in range(3):
                    for t, (c0, c1, _) in enumerate(hchunks):
                        if k == 0:
                            w2_ps[t] = pp1.tile([1, c1 - c0], F32, name="w2ps",
                                                tag="ps1")
                        nc.tensor.matmul(w2_ps[t][:], w2t_sb[k][:, h : h + 1],
                                         h_sb[k, t][:],
                                         start=(k == 0), stop=(k == 2))
                for t, (c0, c1, _) in enumerate(hchunks):
                    o_sb = outp.tile([1, c1 - c0], F32, name="osb", tag="osb")
                    nc.vector.tensor_scalar_add(o_sb[:], w2_ps[t][:],
                                                float(b2_vals[h]))
                    nc.sync.dma_start(y_out[:, c0:c1], o_sb[:])

    nc.finalize()
    return nc


def kernel(action_feats, action_mask, fused, phase_oh,
           We1, be1, We2, be2, W1, b1, W2, b2):
    global LAST_RESULT
    from concourse.bass_utils import run_bass_kernel_spmd

    action_feats = np.asarray(action_feats, dtype=np.float32)
    action_mask = np.asarray(action_mask).astype(bool)
    fused = np.asarray(fused, dtype=np.float32)
    phase_oh = np.asarray(phase_oh, dtype=np.float32)
    We1 = np.asarray(We1, dtype=np.float32)
    be1 = np.asarray(be1, dtype=np.float32)
    We2 = np.asarray(We2, dtype=np.float32)
    be2 = np.asarray(be2, dtype=np.float32)
    W1 = np.asarray(W1, dtype=np.float32)
    b1 = np.asarray(b1, dtype=np.float32)
    W2 = np.asarray(W2, dtype=np.float32)
    b2 = np.asarray(b2, dtype=np.float32)

    head = np.minimum(np.argmax(phase_oh, axis=-1), NH - 1)     # [B]
    kept = ~action_mask                                         # [B, A]
    kept_cnt = kept.sum(axis=1).astype(np.int64)
    kept_a = [np.flatnonzero(kept[b]) for b in range(B)]

    plan = _plan(head, kept_cnt)
    S, G, R = plan["S"], plan["G"], plan["R"]
    slot_b, gco = plan["slot_b"], plan["gco"]

    bias_pack = np.zeros((128, 16), np.float32)
    bias_pack[:, 0] = be1[0:128]
    bias_pack[0:64, 1] = be1[128:192]
    bias_pack[:, 2] = be2[0:128]
    bias_pack[0:64, 3] = be2[128:192]
    for h in range(NH):
        for m in range(3):
            bias_pack[:, 4 + h * 3 + m] = b1[h, 128 * m : 128 * (m + 1)]

    shared = {
        "we1": We1.astype(_DT_NP),
        "we2": We2.astype(_DT_NP),
        "w1e": np.ascontiguousarray(W1[:, :EMB, :]).astype(_DT_NP),
        "w1f": np.ascontiguousarray(W1[:, EMB:, :]).astype(_DT_NP),
        "w2t": np.ascontiguousarray(W2.T).astype(_DT_NP),
        "biaspack": bias_pack,
    }

    in_maps = []
    col_maps = np.full((NCORES, R), -1, np.int64)
    for c in range(NCORES):
        af_t = np.zeros((FEAT, R), _DT_NP)
        s_mat = np.zeros((WIN, R), _DT_NP)
        fu_t = np.zeros((FUS, S), _DT_NP)
        for g in range(G):
            cur = int(gco[g])
            for s in range(WIN * g, WIN * (g + 1)):
                b = slot_b[c, s]
                if b < 0:
                    continue
                fu_t[:, s] = fused[b]
                aa = kept_a[b]
                n = len(aa)
                if n:
                    af_t[:, cur : cur + n] = action_feats[b, aa, :].T
                    s_mat[s - WIN * g, cur : cur + n] = 1.0
                    col_maps[c, cur : cur + n] = b * A + aa
                    cur += n
        in_maps.append({"af_t": af_t, "s_mat": s_mat, "fused_t": fu_t, **shared})

    nc = _build_nc(plan, b2)
    res = run_bass_kernel_spmd(nc, in_maps, core_ids=list(range(NCORES)))
    LAST_RESULT = res

    logits = np.where(action_mask, np.float32(-10000.0),
                      np.float32(0.0)).reshape(-1)
    for c in range(NCORES):
        out = res.results[c]["out"][0]          # [R]
        valid = col_maps[c] >= 0
        logits[col_maps[c][valid]] = out[valid]
    return logits.reshape(B, A)
